# revision 25
# baseline (speedup 1.0000x reference)
"""8-core Trainium2 Bass kernel for nn_BolmoLocalLayer (v2).

Strategy (uniform SPMD program, rank-dependence only in data):
 - host: fold norm1 into Wcat/wv, mh_w into w_out, norm2 into w_gate/w_up;
   pre-transpose x per core; pre-cast weights (and x) to bf16.
 - token-parallel projections (each core: its 512 tokens, all heads);
   payloads staged per m-tile and A2A'd to the head owner.
 - gate projections + gate A2A fired FIRST; the serial gate-prelude math
   (log-sigmoid cumsum, decay columns) overlaps the og/v projections.
 - head-sharded mLSTM attention per batch (m=0 rescale; n=max(|sum C|,1);
   eps*n^2 folded into the per-head RMS norm; per-batch Sigmoid(og) in one
   ACT op; normalization row math vectorized on [8,512] tiles, gpsimd).
 - per-batch A2A of hout (1 MB) replaces the 8 MB ReduceScatter; each core
   computes mix = w_out.T @ hout locally for its own tokens (full w_out
   streamed), adds the residual, norms, fires a per-batch AllGather of h2.
 - MLP FF-sharded, rb-pairs per batch half; batch-0 MLP overlaps batch-1
   wout/x1/AllGather tail.
 - host: scatter x1 rows + sum MLP partials.
"""
import sys

for _p in ("/opt/trn_rl_repo", "/root/.axon_site/_ro/trn_rl_repo"):
    if _p not in sys.path:
        sys.path.append(_p)

import numpy as np
import ml_dtypes

import concourse.mybir as mybir
from concourse import bacc
from concourse.tile import TileContext
from concourse.bass_utils import run_bass_kernel_spmd
from concourse.bass import ds

bf16 = ml_dtypes.bfloat16
FP32 = mybir.dt.float32
BF16 = mybir.dt.bfloat16

B, S, D, H = 2, 2048, 2048, 8
QK, FF = D // 2, 8192
dqk, dv = QK // H, D // H        # 128, 256
R = 8                            # cores
OB = S // R                      # 256 own tokens per batch
OT = 2 * OB                      # 512 own tokens
NK = D // 128                    # 16 contraction tiles over D
CAP, EPS = 15.0, 1e-6
FFC = FF // R                    # 1024 ff slice per core
WCOLS = 2 * QK + D + 2 * H       # 4112
PAY_Q, PAY_K = 0, 65536

AL = mybir.AluOpType
AF = mybir.ActivationFunctionType

_G = {}


def _norm1(nc, xT, xhat, tp, ps1):
    """xhat = rmsnorm(x) for own 512 tokens (norm1_w folded into weights)."""
    ssq_ps = ps1.tile([1, OT], FP32, tag="nrm_row", name="ssq1")
    for kt in range(NK):
        sq = tp.tile([128, OT], BF16, tag="sq", name="sq", bufs=2)
        eng = nc.vector if kt % 2 == 0 else nc.gpsimd
        eng.tensor_tensor(sq[:], xT[:, kt], xT[:, kt], AL.mult)
        nc.tensor.matmul(ssq_ps[:], _G["onesb"][:], sq[:],
                         start=(kt == 0), stop=(kt == NK - 1))
    scl = tp.tile([1, OT], FP32, tag="scl1")
    nc.vector.tensor_scalar(scl[:], ssq_ps[:], 1.0 / D, EPS, AL.mult, AL.add)
    nc.scalar.activation(scl[:], scl[:], AF.Ln)
    nc.scalar.activation(scl[:], scl[:], AF.Exp, scale=-0.5)
    sc_ps = ps1.tile([128, OT], FP32, tag="nrm_bc", name="sc1")
    nc.tensor.matmul(sc_ps[:], _G["ones1f"][0:1, :], scl[:], start=True, stop=True)
    sc_sb = tp.tile([128, OT], FP32, tag="sc_sb", name="sc_sb", bufs=1)
    nc.vector.tensor_copy(sc_sb[:], sc_ps[:])
    for kt in range(NK):
        if kt % 2 == 0:
            nc.vector.tensor_tensor(xhat[:, kt], xT[:, kt], sc_ps[:], AL.mult)
        else:
            nc.gpsimd.tensor_tensor(xhat[:, kt], xT[:, kt], sc_sb[:], AL.mult)


def _proj_m(nc, wp, ps, xhat, wc3, m, mw=128):
    """One mw-col tile of the Wcat projection -> psum."""
    wt = wp.tile([128, NK, 128], BF16, tag="wcat")
    nc.sync.dma_start(wt[:, :, :mw], wc3[:, :, ds(m * 128, mw)])
    pst = ps.tile([128, OT], FP32, tag="proj")
    for kt in range(NK):
        nc.tensor.matmul(pst[:mw, :], wt[:, kt, :mw], xhat[:, kt],
                         start=(kt == 0), stop=(kt == NK - 1))
    return pst


def _gates(nc, pst, b16, bufs, tp):
    gt = tp.tile([16, OT], FP32, tag="gates")
    nc.scalar.activation(gt[:], pst[:16, :], AF.Exp, bias=b16[:],
                         scale=-2.0 / CAP)
    nc.vector.tensor_scalar_add(gt[:], gt[:], 1.0)
    nc.vector.reciprocal(gt[:], gt[:])
    nc.vector.tensor_scalar(gt[:], gt[:], 2.0 * CAP, -CAP, AL.mult, AL.add)
    for hh in range(H):
        nc.scalar.dma_start(bufs["ag_g_in"][hh], gt[ds(2 * hh, 2), :])
    nc.gpsimd.collective_compute(
        "AllToAll", AL.bypass, replica_groups=bufs["rg"],
        ins=[bufs["ag_g_in"][:]], outs=[bufs["ag_g_out"][:]])


def _prelude(nc, bufs, G0, dpad, psq):
    """Gate math for own head: Fcum rows + decay columns, emitted during the
    og/v projections so the serial [1,S] chains hide behind PE work.

    G0 segments (x S): 0 irow_b0 | 1 irow_b1 | 2 frow_b0 | 3 frow_b1 | 4 work
    P2 segments (x S): Fcum_b0 | Fcum_b1
    """
    ag_g_out, P2, dcolT = bufs["ag_g_out"], bufs["P2"], bufs["dcolT"]
    for src in range(R):
        nc.scalar.dma_start(G0[:, ds(0 * S + OB * src, OB)],
                            ag_g_out[src, 0:1, 0:OB])
        nc.scalar.dma_start(G0[:, ds(2 * S + OB * src, OB)],
                            ag_g_out[src, 1:2, 0:OB])
        nc.scalar.dma_start(G0[:, ds(1 * S + OB * (7 - src), OB)],
                            ag_g_out[src, 0:1, OB:OT])
        nc.scalar.dma_start(G0[:, ds(3 * S + OB * (7 - src), OB)],
                            ag_g_out[src, 1:2, OB:OT])
    nc.vector.memset(dpad[:], 0.0)
    for b in range(2):
        wk = G0[:, ds((2 + b) * S, S)]          # in-place over the f row
        nc.scalar.activation(wk, wk, AF.Exp, scale=-1.0)
        nc.vector.tensor_scalar_add(wk, wk, 1.0)
        nc.scalar.activation(wk, wk, AF.Ln)
        nc.vector.tensor_scalar_mul(wk, wk, -1.0)
        nc.vector.tensor_tensor_scan(P2[:, ds(b * S, S)], wk, wk, 0.0,
                                     AL.add, AL.bypass)
        # d = i - Fcum, in place over the i row
        di = G0[:, ds(b * S, S)]
        nc.vector.tensor_tensor(di, di, P2[:, ds(b * S, S)], AL.subtract)
        nc.scalar.dma_start(dpad[b:b + 1, :], di)


def _prelude_cols(nc, bufs, dpad, psq):
    """dcolT extraction; emitted after og so PE is not blocked mid-mixer."""
    dcolT = bufs["dcolT"]
    for ck in range(16):
        tps = psq.tile([128, 128], FP32, tag="tps")
        nc.tensor.transpose(tps[:], dpad[:, ds(ck * 128, 128)], _G["ident"][:])
        nc.vector.tensor_copy(dcolT[:, ck], tps[:, 0:2])


def _vproj(nc, xhat, wv_d, bufs, wvp, tp, ps):
    """v = xhat.T @ wv in 8 head-sized pieces; payload + A2A."""
    wv3 = wv_d.rearrange("(kt p) f -> p kt f", p=128)
    for pp in range(4):          # piece pp = heads (2pp, 2pp+1)
        wvt = wvp.tile([128, NK, 512], BF16, tag="wv")
        nc.sync.dma_start(wvt[:], wv3[:, :, ds(pp * 512, 512)])
        for tt in range(4):      # own-token tiles of 128
            pst = ps.tile([128, 512], FP32, tag="vproj")
            for kt in range(NK):
                nc.tensor.matmul(pst[:], xhat[:, kt, ds(tt * 128, 128)],
                                 wvt[:, kt], start=(kt == 0), stop=(kt == NK - 1))
            vsb = tp.tile([128, 512], BF16, tag="vsb")
            nc.vector.tensor_copy(vsb[:], pst[:])
            for dd in range(2):
                nc.scalar.dma_start(
                    bufs["a2av_in"][2 * pp + dd, ds(tt * 32768, 32768)].rearrange(
                        "(p c) -> p c", p=128), vsb[:, ds(dd * 256, 256)])
    nc.gpsimd.collective_compute(
        "AllToAll", AL.bypass, replica_groups=bufs["rg"],
        ins=[bufs["a2av_in"][:]], outs=[bufs["a2av_out"][:]])


def _mixer(nc, tc, xT, wcat_d, wv_d, b16, bufs):
    rg = bufs["rg"]
    with tc.tile_pool(name="mx_w", bufs=2) as wp, \
         tc.tile_pool(name="mx_wv", bufs=2) as wvp, \
         tc.tile_pool(name="mx_tmp", bufs=3) as tp, \
         tc.tile_pool(name="mx_pre", bufs=1) as pp, \
         tc.tile_pool(name="mx_xh", bufs=1) as xp, \
         tc.tile_pool(name="mx_ps", bufs=2, space="PSUM") as ps, \
         tc.tile_pool(name="mx_ps1", bufs=1, space="PSUM") as ps1:
        xhat = xp.tile([128, NK, OT], BF16)
        _norm1(nc, xT, xhat, tp, ps1)
        wc3 = wcat_d.rearrange("(kt p) f -> p kt f", p=128)

        # gates first -> fire A2A so the prelude can overlap og/v
        pst = _proj_m(nc, wp, ps, xhat, wc3, 32, mw=16)
        _gates(nc, pst, b16, bufs, tp)

        # q (m 0..7) and k (m 8..15); drain straight to a2a1 payload
        for m in range(16):
            pst = _proj_m(nc, wp, ps, xhat, wc3, m)
            st = tp.tile([128, OT], BF16, tag="qkstage")
            nc.vector.tensor_copy(st[:], pst[:])
            off = PAY_Q if m < 8 else PAY_K
            nc.scalar.dma_start(
                bufs["a2a1_in"][m % 8, ds(off, 65536)].rearrange(
                    "(p t) -> p t", p=128), st[:])
        nc.gpsimd.collective_compute(
            "AllToAll", AL.bypass, replica_groups=rg,
            ins=[bufs["a2a1_in"][:]], outs=[bufs["a2a1_out"][:]])

        # prelude (gates A2A has landed by now; overlaps v/og PE work)
        G0 = pp.tile([1, 4 * S], FP32)
        dpad = pp.tile([128, S], FP32)
        _prelude(nc, bufs, G0, dpad, ps)
        del G0

        # og (m 16..31); payload half = m parity
        for m in range(16, 32):
            pst = _proj_m(nc, wp, ps, xhat, wc3, m)
            st = tp.tile([128, OT], BF16, tag="qkstage")
            nc.vector.tensor_copy(st[:], pst[:])
            hh, half = (m - 16) // 2, (m - 16) % 2
            nc.scalar.dma_start(
                bufs["a2aog_in"][hh, ds(half * 65536, 65536)].rearrange(
                    "(p t) -> p t", p=128), st[:])
        nc.gpsimd.collective_compute(
            "AllToAll", AL.bypass, replica_groups=rg,
            ins=[bufs["a2aog_in"][:]], outs=[bufs["a2aog_out"][:]])

        _vproj(nc, xhat, wv_d, bufs, wvp, tp, ps)
        _prelude_cols(nc, bufs, dpad, ps)


def _attn_batch_start(nc, b, bufs, ap):
    """Payload loads + sigb + row tiles for batch b."""
    v_b = ap.tile([128, R, 2, 256], BF16, tag="v_b", name=f"v_b{b}")
    for j in range(2):
        nc.sync.dma_start(
            v_b[:, :, j],
            bufs["a2av_out"][:, ds((2 * b + j) * 32768, 32768)].rearrange(
                "r (p c) -> p r c", p=128))
    hout = ap.tile([128, 2, R, OB], BF16, tag="hout", name=f"hout{b}")
    for half in range(2):
        nc.sync.dma_start(
            hout[:, half],
            bufs["a2aog_out"][:, ds(half * 65536, 65536)].rearrange(
                "r (p t) -> p r t", p=128)[:, :, ds(b * OB, OB)])
    sigb = ap.tile([128, 2, R, OB], BF16, tag="sigb", name=f"sigb{b}")
    nc.scalar.activation(sigb[:], hout[:], AF.Sigmoid)
    bufs[f"v_b{b}"] = v_b
    bufs[f"sigb{b}"] = sigb
    bufs[f"hout{b}"] = hout
    bufs[f"argb{b}"] = ap.tile([128, 4, 512], FP32, tag="argb", name=f"argb{b}")


def _attn_tb(nc, b, tb, bufs, psq, psA, ps1, tp):
    """Group matmuls for (batch b, query block tb); raw A -> hout slots."""
    qT_all, kT_all = bufs["qT_all"], bufs["kT_all"]
    dcolT, P2, strip = bufs["dcolT"], bufs["P2"], _G["strip"]
    v_b, hout, argb = bufs[f"v_b{b}"], bufs[f"hout{b}"], bufs[f"argb{b}"]
    fb_ps = ps1.tile([128, 512], FP32, tag="bc512", name="fb_ps")
    nc.tensor.matmul(fb_ps[:], _G["ones1f"][0:1, :],
                     P2[:, ds(b * S + tb * 512, 512)], start=True, stop=True)
    A0 = psA.tile([128, 512], FP32, tag="A0")
    A1 = psA.tile([128, 512], FP32, tag="A1")
    n_ps = ps1.tile([1, 512], FP32, tag="rowacc", name="n_ps")
    nst = 4 * tb + 4
    qks = []
    for g in range(nst + 1):
        if g < nst:
            src = (g // 2) if b == 0 else (7 - g // 2)
            co = (g % 2) * 128 + b * OB
            qk = psq.tile([128, 512], FP32, tag="qk")
            if b == 0:
                nc.tensor.matmul(qk[:], kT_all[:, src, ds(co, 128)],
                                 qT_all[:, ds(2 * tb, 2), 0:OB],
                                 start=True, stop=True)
            else:
                nc.tensor.matmul(qk[:, 0:256], kT_all[:, src, ds(co, 128)],
                                 qT_all[:, 7 - 2 * tb, OB:OT],
                                 start=True, stop=True)
                nc.tensor.matmul(qk[:, 256:512], kT_all[:, src, ds(co, 128)],
                                 qT_all[:, 6 - 2 * tb, OB:OT],
                                 start=True, stop=True)
            qks.append(qk)
        if g >= 1:               # post-process group g-1 (1-deep pipeline)
            gp = g - 1
            src = (gp // 2) if b == 0 else (7 - gp // 2)
            qkp = qks[gp]
            sexp = tp.tile([128, 512], BF16, tag="sexp")
            nc.scalar.activation(sexp[:], fb_ps[:], AF.Exp,
                                 bias=dcolT[:, gp, b:b + 1])
            cp = tp.tile([128, 512], BF16, tag="cp")
            nc.vector.tensor_tensor(cp[:], qkp[:], sexp[:], AL.mult)
            if gp >= 4 * tb:
                kk = gp - 4 * tb
                nc.vector.tensor_tensor(
                    cp[:], cp[:], strip[:, ds((3 - kk) * 128, 512)], AL.mult)
            nc.tensor.matmul(n_ps[:], _G["onesb"][:], cp[:],
                             start=(gp == 0), stop=(gp == nst - 1))
            vi = gp % 2
            nc.tensor.matmul(A0[:], v_b[:, src, vi, ds(0, 128)], cp[:],
                             start=(gp == 0), stop=(gp == nst - 1))
            nc.tensor.matmul(A1[:], v_b[:, src, vi, ds(128, 128)], cp[:],
                             start=(gp == 0), stop=(gp == nst - 1))
    for j in range(2):
        nc.scalar.activation(hout[:, 0, 2 * tb + j], A0[:, ds(j * 256, 256)],
                             AF.Copy)
        nc.scalar.activation(hout[:, 1, 2 * tb + j], A1[:, ds(j * 256, 256)],
                             AF.Copy)
    # inline normalization arg: arg = ssqA/dv + EPS*max(n^2, 1)
    ssq_ps = ps1.tile([1, 512], FP32, tag="rowacc", name="ssq_ps")
    for half in range(2):
        asq = tp.tile([128, 2, 256], BF16, tag="asq")
        nc.vector.tensor_tensor(asq[:], hout[:, half, ds(2 * tb, 2)],
                                hout[:, half, ds(2 * tb, 2)], AL.mult)
        nc.tensor.matmul(ssq_ps[:], _G["onesb"][:], asq[:],
                         start=(half == 0), stop=(half == 1))
    nrow_sb = tp.tile([1, 512], FP32, tag="nrow_sb")
    nc.vector.tensor_copy(nrow_sb[:], n_ps[:])
    nc.vector.tensor_tensor(nrow_sb[:], nrow_sb[:], nrow_sb[:], AL.mult)
    nc.vector.tensor_scalar(nrow_sb[:], nrow_sb[:], 1.0, EPS, AL.max, AL.mult)
    srow_sb = tp.tile([1, 512], FP32, tag="srow_sb")
    nc.vector.tensor_copy(srow_sb[:], ssq_ps[:])
    nb = psA.tile([128, 512], FP32, tag="A0", name="nb")
    nc.tensor.matmul(nb[:], _G["ones1f"][0:1, :], nrow_sb[:],
                     start=True, stop=True)
    sb2 = psA.tile([128, 512], FP32, tag="A1", name="sb2")
    nc.tensor.matmul(sb2[:], _G["ones1f"][0:1, :], srow_sb[:],
                     start=True, stop=True)
    s2 = tp.tile([128, 512], FP32, tag="s2")
    nc.scalar.activation(s2[:], sb2[:], AF.Copy, scale=1.0 / dv)
    nc.vector.tensor_tensor(argb[:, tb], nb[:], s2[:], AL.add)


def _attn_batch_tail(nc, b, bufs, ps1, tp):
    _attn_tail_compute(nc, b, bufs, tp)
    _attn_tail_fire(nc, b, bufs)


def _attn_tail_compute(nc, b, bufs, tp):
    """Bulk rsqrt + gate/scale hout (no collective interaction)."""
    hout, argb = bufs[f"hout{b}"], bufs[f"argb{b}"]
    sigb = bufs[f"sigb{b}"]
    # srow = rsqrt(arg), all 4 tb in two ACT ops (in place)
    nc.scalar.activation(argb[:], argb[:], AF.Ln)
    nc.scalar.activation(argb[:], argb[:], AF.Exp, scale=-0.5)
    for tb in range(4):
        for half in range(2):
            for j in range(2):
                sg = (2 * tb + j) if b == 0 else (7 - 2 * tb - j)
                hs = hout[:, half, 2 * tb + j]
                nc.vector.tensor_tensor(hs, hs, sigb[:, half, sg], AL.mult)
                nc.vector.tensor_tensor(hs, hs, argb[:, tb, ds(j * 256, 256)],
                                        AL.mult)


def _attn_tail_fire(nc, b, bufs):
    """Payload stores + the hout A2A."""
    rg = bufs["rg"]
    hout = bufs[f"hout{b}"]
    for tb in range(4):
        for hf in range(2):
            dest = (2 * tb + hf) if b == 0 else (7 - 2 * tb - hf)
            nc.scalar.dma_start(
                bufs["a2ah_in"][b][dest].rearrange(
                    "(p half t) -> p half t", p=128, half=2),
                hout[:, :, 2 * tb + hf])
    nc.gpsimd.collective_compute(
        "AllToAll", AL.bypass, replica_groups=rg,
        ins=[bufs["a2ah_in"][b][:]], outs=[bufs["a2ah_out"][b][:]])


def _wout_x1_pieces(nc, b, bufs, wx, wps, mlo, mhi, tp):
    """w_out mix + residual + norm2-ssq for batch-b tokens, m in [mlo,mhi)."""
    if mlo == 0:
        hA = wx.tile([128, 8, 2, 256], BF16, tag="hA", name=f"hA{b}")
        bufs["hA_cur"] = hA
        nc.sync.dma_start(
            hA[:],
            bufs["a2ah_out"][b].rearrange("r (p half t) -> p r half t",
                                          p=128, half=2))
        bufs["x1b_cur"] = wx.tile([128, NK, OB], BF16, tag="x1b", name=f"x1b{b}")
        bufs["x1row_ps"] = wps.tile([1, OB], FP32, tag="x1row", name="x1row", bufs=1)
    hA, x1b = bufs["hA_cur"], bufs["x1b_cur"]
    wo3 = bufs["wout_d"].rearrange("(kt p) f -> p kt f", p=128)
    for m in range(mlo, mhi):
        wpc = wx.tile([128, 16, 128], BF16, tag="wopc", bufs=2)
        nc.sync.dma_start(wpc[:], wo3[:, :, ds(m * 128, 128)])
        mps = wps.tile([128, OB], FP32, tag="wops", name="mps", bufs=1)
        for kt in range(16):
            nc.tensor.matmul(mps[:], wpc[:, kt], hA[:, kt // 2, kt % 2],
                             start=(kt == 0), stop=(kt == 15),
                             skip_group_check=True)
        nc.vector.tensor_tensor(x1b[:, m], bufs["xT"][:, m, ds(b * OB, OB)],
                                mps[:], AL.add)
        sqt = tp.tile([128, OB], BF16, tag="sqx")
        nc.vector.tensor_tensor(sqt[:], x1b[:, m], x1b[:, m], AL.mult)
        nc.tensor.matmul(bufs["x1row_ps"][:], _G["onesb"][:], sqt[:],
                         start=(m == 0), stop=(m == 15),
                         skip_group_check=True)


def _wout_x1_tail(nc, b, bufs, wx, wps, tp):
    x1b = bufs["x1b_cur"]
    nc.scalar.dma_start(
        bufs["ox1_d"].rearrange("(kt p) t -> p kt t", p=128)[:, :, ds(b * OB, OB)],
        x1b[:])
    scl = tp.tile([1, OB], FP32, tag="sclx")
    nc.vector.tensor_scalar(scl[:], bufs["x1row_ps"][:], 1.0 / D, EPS,
                            AL.mult, AL.add)
    nc.scalar.activation(scl[:], scl[:], AF.Ln)
    nc.scalar.activation(scl[:], scl[:], AF.Exp, scale=-0.5)
    sc_ps = wps.tile([128, OB], FP32, tag="wops", name="sc_ps", bufs=1)
    nc.tensor.matmul(sc_ps[:], _G["ones1f"][0:1, :], scl[:], start=True, stop=True)
    h2T = wx.tile([128, NK, OB], BF16, tag="h2T", name=f"h2T{b}")
    for kt in range(NK):
        nc.vector.tensor_tensor(h2T[:, kt], x1b[:, kt], sc_ps[:], AL.mult)
    nc.sync.dma_start(
        bufs["ag2_in"][b].rearrange("(kt p) t -> p kt t", p=128), h2T[:])
    nc.gpsimd.collective_compute(
        "AllGather", AL.bypass, replica_groups=bufs["rg"],
        ins=[bufs["ag2_in"][b][:]], outs=[bufs["ag2_out"][b][:]])


def _mlp_pair(nc, b, pair, bufs, hp, tp, psgu, psd):
    """MLP for rb blocks (2*pair, 2*pair+1), batch-b half (2x256 tokens)."""
    h2b = hp.tile([128, NK, OT], BF16, tag="h2b", bufs=2,
                  name=f"h2b_{b}_{pair}")
    for j in range(2):
        nc.sync.dma_start(
            h2b[:, :, ds(j * OB, OB)],
            bufs["ag2_out"][b][2 * pair + j].rearrange("(kt p) t -> p kt t",
                                                       p=128))
    if "wd_t" not in bufs:
        bufs["wd_t"] = hp.tile([128, 8, D], BF16, name="wd_t")
        nc.sync.dma_start(bufs["wd_t"][:],
                          bufs["wd_d"].rearrange("(kt p) f -> p kt f", p=128))
    wg_t, wu_t, wd_t = bufs["wg_t"], bufs["wu_t"], bufs["wd_t"]
    ga = hp.tile([128, 8, OT], BF16, tag="ga", name=f"ga_{b}_{pair}")
    for mf in range(8):
        gps = psgu.tile([128, OT], FP32, tag="g")
        for kt in range(NK):
            nc.tensor.matmul(gps[:], wg_t[:, kt, ds(mf * 128, 128)],
                             h2b[:, kt], start=(kt == 0), stop=(kt == NK - 1))
        nc.scalar.activation(ga[:, mf], gps[:], AF.Silu)
        ups = psgu.tile([128, OT], FP32, tag="u")
        for kt in range(NK):
            nc.tensor.matmul(ups[:], wu_t[:, kt, ds(mf * 128, 128)],
                             h2b[:, kt], start=(kt == 0), stop=(kt == NK - 1))
        # aa = silu(g) * u written in place over ga
        nc.vector.tensor_tensor(ga[:, mf], ups[:], ga[:, mf], AL.mult)
    for tt in range(4):
        rb = 2 * pair + tt // 2
        row0 = rb * OT + b * OB + (tt % 2) * 128
        opss = [psd.tile([128, 512], FP32, tag=f"o{nb}", name=f"o{nb}")
                for nb in range(4)]
        for kt in range(8):
            for nb in range(4):
                nc.tensor.matmul(opss[nb][:], ga[:, kt, ds(tt * 128, 128)],
                                 wd_t[:, kt, ds(nb * 512, 512)],
                                 start=(kt == 0), stop=(kt == 7),
                                 skip_group_check=True)
        for nb in range(4):
            ost = tp.tile([128, 512], BF16, tag="ost", name="ost", bufs=1)
            nc.scalar.activation(ost[:], opss[nb][:], AF.Copy)
            nc.scalar.dma_start(
                bufs["omlp_d"][ds(row0, 128), ds(nb * 512, 512)], ost[:])


def _build():
    nc = bacc.Bacc(num_devices=R)
    rg = [list(range(R))]

    xT_d = nc.dram_tensor("xT", [D, OT], BF16, kind="ExternalInput")
    wcat_d = nc.dram_tensor("wcat", [D, WCOLS], BF16, kind="ExternalInput")
    wv_d = nc.dram_tensor("wv", [D, D], BF16, kind="ExternalInput")
    b16_d = nc.dram_tensor("b16", [16, 1], FP32, kind="ExternalInput")
    wout_d = nc.dram_tensor("wout", [D, D], BF16, kind="ExternalInput")
    wg_d = nc.dram_tensor("wg", [D, FFC], BF16, kind="ExternalInput")
    wu_d = nc.dram_tensor("wu", [D, FFC], BF16, kind="ExternalInput")
    wd_d = nc.dram_tensor("wd", [FFC, D], BF16, kind="ExternalInput")
    strip_d = nc.dram_tensor("strip", [128, 896], BF16, kind="ExternalInput")
    ident_d = nc.dram_tensor("ident", [128, 128], FP32, kind="ExternalInput")
    ones1f_d = nc.dram_tensor("ones1f", [65, 128], FP32, kind="ExternalInput")
    onesb_d = nc.dram_tensor("onesb", [128, 1], BF16, kind="ExternalInput")

    ox1_d = nc.dram_tensor("out_x1", [D, OT], BF16, kind="ExternalOutput")
    omlp_d = nc.dram_tensor("out_mlp", [R * OT, D], BF16, kind="ExternalOutput")

    bufs = {
        "rg": rg, "ox1_d": ox1_d, "omlp_d": omlp_d, "wout_d": wout_d,
        "wg_d": wg_d, "wu_d": wu_d, "wd_d": wd_d,
        "a2a1_in": nc.dram_tensor("a2a1_in", [R, 131072], BF16),
        "a2a1_out": nc.dram_tensor("a2a1_out", [R, 131072], BF16),
        "a2av_in": nc.dram_tensor("a2av_in", [R, 131072], BF16),
        "a2av_out": nc.dram_tensor("a2av_out", [R, 131072], BF16),
        "a2aog_in": nc.dram_tensor("a2aog_in", [R, 131072], BF16),
        "a2aog_out": nc.dram_tensor("a2aog_out", [R, 131072], BF16),
        "ag_g_in": nc.dram_tensor("ag_g_in", [R, 2, OT], FP32),
        "ag_g_out": nc.dram_tensor("ag_g_out", [R, 2, OT], FP32),
        "a2ah_in": [nc.dram_tensor(f"a2ah_in{b}", [R, 65536], BF16)
                    for b in range(2)],
        "a2ah_out": [nc.dram_tensor(f"a2ah_out{b}", [R, 65536], BF16)
                     for b in range(2)],
        "ag2_in": [nc.dram_tensor(f"ag2_in{b}", [D, OB], BF16)
                   for b in range(2)],
        "ag2_out": [nc.dram_tensor(f"ag2_out{b}", [R, D, OB], BF16,
                                   addr_space="Shared") for b in range(2)],
    }

    with TileContext(nc) as tc:
        with tc.tile_pool(name="glob", bufs=1) as gp:
            strip = gp.tile([128, 896], BF16)
            nc.sync.dma_start(strip[:], strip_d[:])
            ident = gp.tile([128, 128], FP32)
            nc.sync.dma_start(ident[:], ident_d[:])
            ones1f = gp.tile([65, 128], FP32)
            nc.sync.dma_start(ones1f[:], ones1f_d[:])
            onesb = gp.tile([128, 1], BF16)
            nc.sync.dma_start(onesb[:], onesb_d[:])
            b16 = gp.tile([16, 1], FP32)
            nc.sync.dma_start(b16[:], b16_d[:])
            _G.update(strip=strip, ident=ident, ones1f=ones1f, onesb=onesb)

            # HAM warmup: junk matmuls while xT/weights stream in
            with tc.tile_pool(name="warm", bufs=2, space="PSUM") as wmp:
                for _ in range(24):
                    wps0 = wmp.tile([128, 512], FP32, tag="wm")
                    nc.tensor.matmul(wps0[:], strip[:, 0:128], strip[:, 128:640],
                                     start=True, stop=True)

            with tc.tile_pool(name="mid", bufs=1) as mp:
                xT = mp.tile([128, NK, OT], BF16)
                nc.sync.dma_start(xT[:],
                                  xT_d.rearrange("(kt p) t -> p kt t", p=128))
                bufs["xT"] = xT
                bufs["P2"] = mp.tile([1, 2 * S], FP32, name="P2")
                bufs["dcolT"] = mp.tile([128, 16, 2], FP32, name="dcolT")

                _mixer(nc, tc, xT, wcat_d, wv_d, b16, bufs)

                with tc.tile_pool(name="mlpw", bufs=1) as mwp, \
                     tc.tile_pool(name="wx", bufs=1) as wx, \
                     tc.tile_pool(name="wx_tp", bufs=2) as wxtp, \
                     tc.tile_pool(name="wx_ps", bufs=2, space="PSUM") as wps:
                    with tc.tile_pool(name="at_pay", bufs=1) as ap, \
                         tc.tile_pool(name="at_tmp", bufs=2) as tp, \
                         tc.tile_pool(name="at_psq", bufs=2, space="PSUM") as psq, \
                         tc.tile_pool(name="at_psA", bufs=1, space="PSUM") as psA, \
                         tc.tile_pool(name="at_ps1", bufs=1, space="PSUM") as ps1:
                        qT_all = ap.tile([128, R, OT], BF16)
                        nc.sync.dma_start(
                            qT_all[:],
                            bufs["a2a1_out"][:, ds(PAY_Q, 65536)].rearrange(
                                "r (p t) -> p r t", p=128))
                        kT_all = ap.tile([128, R, OT], BF16)
                        nc.sync.dma_start(
                            kT_all[:],
                            bufs["a2a1_out"][:, ds(PAY_K, 65536)].rearrange(
                                "r (p t) -> p r t", p=128))
                        bufs["qT_all"], bufs["kT_all"] = qT_all, kT_all
                        bufs["wg_t"] = mwp.tile([128, NK, FFC], BF16,
                                                name="wg_t")
                        bufs["wu_t"] = mwp.tile([128, NK, FFC], BF16,
                                                name="wu_t")
                        nc.sync.dma_start(
                            bufs["wg_t"][:],
                            bufs["wg_d"].rearrange("(kt p) f -> p kt f", p=128))
                        nc.sync.dma_start(
                            bufs["wu_t"][:],
                            bufs["wu_d"].rearrange("(kt p) f -> p kt f", p=128))

                        _attn_batch_start(nc, 0, bufs, ap)
                        for tb in range(4):
                            _attn_tb(nc, 0, tb, bufs, psq, psA, ps1, tp)
                        _attn_tail_compute(nc, 0, bufs, tp)
                        # b1 payload loads go on the gpsimd queue BEFORE the
                        # hout A2A trigger so they don't wait on it
                        _attn_batch_start(nc, 1, bufs, ap)
                        _attn_tail_fire(nc, 0, bufs)
                        _attn_tb(nc, 1, 0, bufs, psq, psA, ps1, tp)
                        _wout_x1_pieces(nc, 0, bufs, wx, wps, 0, 8, wxtp)
                        _attn_tb(nc, 1, 1, bufs, psq, psA, ps1, tp)
                        _wout_x1_pieces(nc, 0, bufs, wx, wps, 8, 16, wxtp)
                        _wout_x1_tail(nc, 0, bufs, wx, wps, wxtp)
                        _attn_tb(nc, 1, 2, bufs, psq, psA, ps1, tp)
                        _attn_tb(nc, 1, 3, bufs, psq, psA, ps1, tp)
                        _attn_tail_compute(nc, 1, bufs, tp)
                        _attn_tail_fire(nc, 1, bufs)

                    with tc.tile_pool(name="ml_h", bufs=1) as hp, \
                         tc.tile_pool(name="ml_tmp", bufs=2) as mtp, \
                         tc.tile_pool(name="ml_psgu", bufs=1, space="PSUM") as psgu, \
                         tc.tile_pool(name="ml_psd", bufs=1, space="PSUM") as psd:
                        _mlp_pair(nc, 0, 0, bufs, hp, mtp, psgu, psd)
                        _wout_x1_pieces(nc, 1, bufs, wx, wps, 0, 16, wxtp)
                        _wout_x1_tail(nc, 1, bufs, wx, wps, wxtp)
                        for pair in range(1, 4):
                            _mlp_pair(nc, 0, pair, bufs, hp, mtp, psgu, psd)
                        for pair in range(4):
                            _mlp_pair(nc, 1, pair, bufs, hp, mtp, psgu, psd)

    nc.finalize()
    return nc


_NC_CACHE = None


def kernel(x, norm1_w, wq, wk, wv, w_ig, b_ig, w_fg, b_fg, w_og, mh_w,
           w_out, norm2_w, w_gate, w_up, w_down):
    global _NC_CACHE
    x = np.asarray(x, np.float32)
    n1 = np.asarray(norm1_w, np.float32)
    n2 = np.asarray(norm2_w, np.float32)
    mh = np.asarray(mh_w, np.float32)

    wif = np.empty((D, 2 * H), np.float32)
    wif[:, 0::2] = np.asarray(w_ig)
    wif[:, 1::2] = np.asarray(w_fg)
    b16v = np.empty((16, 1), np.float32)
    b16v[0::2, 0] = -2.0 * np.asarray(b_ig) / CAP
    b16v[1::2, 0] = -2.0 * np.asarray(b_fg) / CAP

    wq_s = np.asarray(wq) / np.float32(np.sqrt(dqk))
    wcat = (np.concatenate([wq_s, np.asarray(wk), np.asarray(w_og), wif],
                           axis=1) * n1[:, None]).astype(bf16)
    wv_b = (np.asarray(wv) * n1[:, None]).astype(bf16)
    wout_f = (np.asarray(w_out) * mh[:, None]).astype(bf16)
    wg_f = (np.asarray(w_gate) * n2[:, None]).astype(bf16)
    wu_f = (np.asarray(w_up) * n2[:, None]).astype(bf16)
    wd_b = np.asarray(w_down).astype(bf16)

    i_idx = np.arange(128)[:, None]
    c_idx = np.arange(896)[None, :]
    strip = ((c_idx - i_idx) >= 384).astype(bf16)
    ident = np.eye(128, dtype=np.float32)
    ones1f = np.ones((65, 128), np.float32)
    onesb = np.ones((128, 1), bf16)

    in_maps = []
    for c in range(R):
        s0 = slice(OB * c, OB * (c + 1))
        s1 = slice(OB * (7 - c), OB * (8 - c))
        xT = np.ascontiguousarray(
            np.concatenate([x[0, s0].T, x[1, s1].T], axis=1)).astype(bf16)
        in_maps.append({
            "xT": xT, "wcat": wcat, "wv": wv_b, "b16": b16v,
            "wout": wout_f,
            "wg": np.ascontiguousarray(wg_f[:, FFC * c:FFC * (c + 1)]),
            "wu": np.ascontiguousarray(wu_f[:, FFC * c:FFC * (c + 1)]),
            "wd": np.ascontiguousarray(wd_b[FFC * c:FFC * (c + 1)]),
            "strip": strip, "ident": ident, "ones1f": ones1f, "onesb": onesb,
        })

    if _NC_CACHE is None:
        _NC_CACHE = _build()
    res = run_bass_kernel_spmd(_NC_CACHE, in_maps, core_ids=list(range(R)))

    import os
    if os.environ.get("KDBG"):
        np.savez("/tmp/kdbg.npz",
                 **{f"x1_{c}": np.asarray(res.results[c]["out_x1"])
                    for c in range(R)},
                 **{f"mlp_{c}": np.asarray(res.results[c]["out_mlp"])
                    for c in range(R)})
    out = np.zeros((B, S, D), np.float32)
    for c in range(R):
        x1T = np.asarray(res.results[c]["out_x1"]).astype(np.float32)
        s0 = slice(OB * c, OB * (c + 1))
        s1 = slice(OB * (7 - c), OB * (8 - c))
        out[0, s0] = x1T[:, :OB].T
        out[1, s1] = x1T[:, OB:].T
    mlp = np.zeros((R * OT, D), np.float32)
    for c in range(R):
        mlp += np.asarray(res.results[c]["out_mlp"]).astype(np.float32)
    for r in range(R):
        blk = mlp[r * OT:(r + 1) * OT]
        out[0, OB * r:OB * (r + 1)] += blk[:OB]
        out[1, OB * (7 - r):OB * (8 - r)] += blk[OB:]
    return out


# revision 30
# speedup vs baseline: 1.0736x; 1.0736x over previous
"""8-core Trainium2 Bass kernel for nn_BolmoLocalLayer (v2).

Strategy (uniform SPMD program, rank-dependence only in data):
 - host: fold norm1 into Wcat/wv, mh_w into w_out, norm2 into w_gate/w_up;
   pre-transpose x per core; pre-cast weights (and x) to bf16.
 - token-parallel projections (each core: its 512 tokens, all heads);
   payloads staged per m-tile and A2A'd to the head owner.
 - gate projections + gate A2A fired FIRST; the serial gate-prelude math
   (log-sigmoid cumsum, decay columns) overlaps the og/v projections.
 - head-sharded mLSTM attention per batch (m=0 rescale; n=max(|sum C|,1);
   eps*n^2 folded into the per-head RMS norm; per-batch Sigmoid(og) in one
   ACT op; normalization row math vectorized on [8,512] tiles, gpsimd).
 - per-batch A2A of hout (1 MB) replaces the 8 MB ReduceScatter; each core
   computes mix = w_out.T @ hout locally for its own tokens (full w_out
   streamed), adds the residual, norms, fires a per-batch AllGather of h2.
 - MLP FF-sharded, rb-pairs per batch half; batch-0 MLP overlaps batch-1
   wout/x1/AllGather tail.
 - host: scatter x1 rows + sum MLP partials.
"""
import sys

for _p in ("/opt/trn_rl_repo", "/root/.axon_site/_ro/trn_rl_repo"):
    if _p not in sys.path:
        sys.path.append(_p)

import numpy as np
import ml_dtypes

import concourse.mybir as mybir
from concourse import bacc
from concourse.tile import TileContext
from concourse.bass_utils import run_bass_kernel_spmd
from concourse.bass import ds

bf16 = ml_dtypes.bfloat16
FP32 = mybir.dt.float32
BF16 = mybir.dt.bfloat16

B, S, D, H = 2, 2048, 2048, 8
QK, FF = D // 2, 8192
dqk, dv = QK // H, D // H        # 128, 256
R = 8                            # cores
OB = S // R                      # 256 own tokens per batch
OT = 2 * OB                      # 512 own tokens
NK = D // 128                    # 16 contraction tiles over D
CAP, EPS = 15.0, 1e-6
FFC = FF // R                    # 1024 ff slice per core
WCOLS = 2 * QK + D + 2 * H       # 4112
PAY_Q, PAY_K = 0, 65536

AL = mybir.AluOpType
AF = mybir.ActivationFunctionType

_G = {}


def _norm1(nc, xT, xhat, tp, ps1):
    """xhat = rmsnorm(x) for own 512 tokens (norm1_w folded into weights)."""
    ssq_ps = ps1.tile([1, OT], FP32, tag="nrm_row", name="ssq1")
    for kt in range(NK):
        sq = tp.tile([128, OT], BF16, tag="sq", name="sq", bufs=2)
        eng = nc.vector if kt % 2 == 0 else nc.gpsimd
        eng.tensor_tensor(sq[:], xT[:, kt], xT[:, kt], AL.mult)
        nc.tensor.matmul(ssq_ps[:], _G["onesb"][:], sq[:],
                         start=(kt == 0), stop=(kt == NK - 1))
    scl = tp.tile([1, OT], FP32, tag="scl1")
    nc.vector.tensor_scalar(scl[:], ssq_ps[:], 1.0 / D, EPS, AL.mult, AL.add)
    nc.scalar.activation(scl[:], scl[:], AF.Ln)
    nc.scalar.activation(scl[:], scl[:], AF.Exp, scale=-0.5)
    sc_ps = ps1.tile([128, OT], FP32, tag="nrm_bc", name="sc1")
    nc.tensor.matmul(sc_ps[:], _G["ones1f"][0:1, :], scl[:], start=True, stop=True)
    sc_sb = tp.tile([128, OT], FP32, tag="sc_sb", name="sc_sb", bufs=1)
    nc.vector.tensor_copy(sc_sb[:], sc_ps[:])
    for kt in range(NK):
        if kt % 2 == 0:
            nc.vector.tensor_tensor(xhat[:, kt], xT[:, kt], sc_ps[:], AL.mult)
        else:
            nc.gpsimd.tensor_tensor(xhat[:, kt], xT[:, kt], sc_sb[:], AL.mult)


def _proj_m(nc, wp, ps, xhat, wc3, m, mw=128):
    """One mw-col tile of the Wcat projection -> psum."""
    wt = wp.tile([128, NK, 128], BF16, tag="wcat")
    nc.sync.dma_start(wt[:, :, :mw], wc3[:, :, ds(m * 128, mw)])
    pst = ps.tile([128, OT], FP32, tag="proj")
    for kt in range(NK):
        nc.tensor.matmul(pst[:mw, :], wt[:, kt, :mw], xhat[:, kt],
                         start=(kt == 0), stop=(kt == NK - 1))
    return pst


def _gates(nc, pst, b16, bufs, tp):
    gt = tp.tile([16, OT], FP32, tag="gates")
    nc.scalar.activation(gt[:], pst[:16, :], AF.Exp, bias=b16[:],
                         scale=-2.0 / CAP)
    nc.vector.tensor_scalar_add(gt[:], gt[:], 1.0)
    nc.vector.reciprocal(gt[:], gt[:])
    nc.vector.tensor_scalar(gt[:], gt[:], 2.0 * CAP, -CAP, AL.mult, AL.add)
    for hh in range(H):
        nc.scalar.dma_start(bufs["ag_g_in"][hh], gt[ds(2 * hh, 2), :])
    nc.gpsimd.collective_compute(
        "AllToAll", AL.bypass, replica_groups=bufs["rg"],
        ins=[bufs["ag_g_in"][:]], outs=[bufs["ag_g_out"][:]])


def _prelude_steps(nc, bufs, G0, dpad):
    """Gate math for own head as a list of emission steps (interleaved into
    the v/og loops so the serial [1,S] chain never blocks the psum drains).

    G0 segments (x S): 0 irow_b0 | 1 irow_b1 | 2 frow_b0 | 3 frow_b1
    P2 segments (x S): Fcum_b0 | Fcum_b1
    """
    ag_g_out, P2 = bufs["ag_g_out"], bufs["P2"]

    def gathers():
        for src in range(R):
            nc.scalar.dma_start(G0[:, ds(0 * S + OB * src, OB)],
                                ag_g_out[src, 0:1, 0:OB])
            nc.scalar.dma_start(G0[:, ds(2 * S + OB * src, OB)],
                                ag_g_out[src, 1:2, 0:OB])
            nc.scalar.dma_start(G0[:, ds(1 * S + OB * (7 - src), OB)],
                                ag_g_out[src, 0:1, OB:OT])
            nc.scalar.dma_start(G0[:, ds(3 * S + OB * (7 - src), OB)],
                                ag_g_out[src, 1:2, OB:OT])
        nc.vector.memset(dpad[:], 0.0)

    steps = [gathers]
    for b in range(2):
        wk = G0[:, ds((2 + b) * S, S)]          # in-place over the f row
        di = G0[:, ds(b * S, S)]
        Pb = P2[:, ds(b * S, S)]
        steps += [
            (lambda wk=wk: nc.scalar.activation(wk, wk, AF.Exp, scale=-1.0)),
            (lambda wk=wk: nc.vector.tensor_scalar_add(wk, wk, 1.0)),
            (lambda wk=wk: nc.scalar.activation(wk, wk, AF.Ln)),
            (lambda wk=wk: nc.vector.tensor_scalar_mul(wk, wk, -1.0)),
            (lambda wk=wk, Pb=Pb: nc.vector.tensor_tensor_scan(
                Pb, wk, wk, 0.0, AL.add, AL.bypass)),
            (lambda di=di, Pb=Pb: nc.vector.tensor_tensor(
                di, di, Pb, AL.subtract)),
            (lambda b=b, di=di: nc.scalar.dma_start(dpad[b:b + 1, :], di)),
        ]
    return steps


def _prelude_cols(nc, bufs, dpad, psq):
    """dcolT extraction; emitted after og so PE is not blocked mid-mixer."""
    dcolT = bufs["dcolT"]
    for ck in range(16):
        tps = psq.tile([128, 128], FP32, tag="tps")
        nc.tensor.transpose(tps[:], dpad[:, ds(ck * 128, 128)], _G["ident"][:])
        nc.vector.tensor_copy(dcolT[:, ck], tps[:, 0:2])


def _vproj(nc, xhat, wv_d, bufs, wvp, tp, ps, chain):
    """v = xhat.T @ wv in 4 2-head pieces; payload + A2A."""
    wv3 = wv_d.rearrange("(kt p) f -> p kt f", p=128)
    for pp in range(4):          # piece pp = heads (2pp, 2pp+1)
        wvt = wvp.tile([128, NK, 512], BF16, tag="wv")
        nc.sync.dma_start(wvt[:], wv3[:, :, ds(pp * 512, 512)])
        for tt in range(4):      # own-token tiles of 128
            pst = ps.tile([128, 512], FP32, tag="vproj")
            for kt in range(NK):
                nc.tensor.matmul(pst[:], xhat[:, kt, ds(tt * 128, 128)],
                                 wvt[:, kt], start=(kt == 0), stop=(kt == NK - 1))
            vsb = tp.tile([128, 512], BF16, tag="vsb")
            nc.vector.tensor_copy(vsb[:], pst[:])
            for dd in range(2):
                nc.scalar.dma_start(
                    bufs["a2av_in"][2 * pp + dd, ds(tt * 32768, 32768)].rearrange(
                        "(p c) -> p c", p=128), vsb[:, ds(dd * 256, 256)])
            if chain:
                chain.pop(0)()
    nc.gpsimd.collective_compute(
        "AllToAll", AL.bypass, replica_groups=bufs["rg"],
        ins=[bufs["a2av_in"][:]], outs=[bufs["a2av_out"][:]])


def _mixer(nc, tc, xT, wcat_d, wv_d, b16, bufs):
    rg = bufs["rg"]
    with tc.tile_pool(name="mx_w", bufs=2) as wp, \
         tc.tile_pool(name="mx_wv", bufs=2) as wvp, \
         tc.tile_pool(name="mx_tmp", bufs=3) as tp, \
         tc.tile_pool(name="mx_pre", bufs=1) as pp, \
         tc.tile_pool(name="mx_xh", bufs=1) as xp, \
         tc.tile_pool(name="mx_ps", bufs=2, space="PSUM") as ps, \
         tc.tile_pool(name="mx_ps1", bufs=1, space="PSUM") as ps1:
        xhat = xp.tile([128, NK, OT], BF16)
        _norm1(nc, xT, xhat, tp, ps1)
        wc3 = wcat_d.rearrange("(kt p) f -> p kt f", p=128)

        # gates first -> fire A2A so the prelude can overlap og/v
        pst = _proj_m(nc, wp, ps, xhat, wc3, 32, mw=16)
        _gates(nc, pst, b16, bufs, tp)

        # q (m 0..7) and k (m 8..15); drain straight to a2a1 payload
        for m in range(16):
            pst = _proj_m(nc, wp, ps, xhat, wc3, m)
            st = tp.tile([128, OT], BF16, tag="qkstage")
            nc.vector.tensor_copy(st[:], pst[:])
            off = PAY_Q if m < 8 else PAY_K
            nc.scalar.dma_start(
                bufs["a2a1_in"][m % 8, ds(off, 65536)].rearrange(
                    "(p t) -> p t", p=128), st[:])
        nc.gpsimd.collective_compute(
            "AllToAll", AL.bypass, replica_groups=rg,
            ins=[bufs["a2a1_in"][:]], outs=[bufs["a2a1_out"][:]])

        # gate-prelude chain as a list of steps, interleaved into the v/og
        # loops so the serial [1,S] latency never blocks the psum drains
        G0 = pp.tile([1, 4 * S], FP32)
        dpad = pp.tile([128, S], FP32)
        chain = _prelude_steps(nc, bufs, G0, dpad)

        _vproj(nc, xhat, wv_d, bufs, wvp, tp, ps, chain)

        # og (m 16..31) -> local sigmoid gate (no collective; the receiver of
        # the hout A2A is the token owner, which is us)
        sigl = bufs["sigl"]
        for m in range(16, 32):
            pst = _proj_m(nc, wp, ps, xhat, wc3, m)
            nc.vector.tensor_copy(sigl[:, (m - 16) // 2, (m - 16) % 2], pst[:])
            if chain:
                chain.pop(0)()
        while chain:
            chain.pop(0)()
        nc.scalar.activation(sigl[:], sigl[:], AF.Sigmoid)
        _prelude_cols(nc, bufs, dpad, ps)


def _attn_batch_start(nc, b, bufs, ap):
    """Payload loads + slot/arg tiles for batch b."""
    v_b = ap.tile([128, R, 2, 256], BF16, tag="v_b", name=f"v_b{b}")
    for j in range(2):
        nc.scalar.dma_start(
            v_b[:, :, j],
            bufs["a2av_out"][:, ds((2 * b + j) * 32768, 32768)].rearrange(
                "r (p c) -> p r c", p=128))
    bufs[f"v_b{b}"] = v_b
    bufs[f"hout{b}"] = ap.tile([128, 2, R, OB], BF16, tag="hout",
                               name=f"hout{b}")
    bufs[f"argb{b}"] = ap.tile([128, 4, 512], FP32, tag="argb", name=f"argb{b}")


def _attn_tb(nc, b, tb, bufs, psq, psA, ps1, tp):
    """Group matmuls for (batch b, query block tb); raw A -> hout slots."""
    qT_all, kT_all = bufs["qT_all"], bufs["kT_all"]
    dcolT, P2, strip = bufs["dcolT"], bufs["P2"], _G["strip"]
    v_b, hout, argb = bufs[f"v_b{b}"], bufs[f"hout{b}"], bufs[f"argb{b}"]
    fb_ps = ps1.tile([128, 512], FP32, tag="bc512", name="fb_ps")
    nc.tensor.matmul(fb_ps[:], _G["ones1f"][0:1, :],
                     P2[:, ds(b * S + tb * 512, 512)], start=True, stop=True)
    A0 = psA.tile([128, 512], FP32, tag="A0")
    A1 = psA.tile([128, 512], FP32, tag="A1")
    n_ps = ps1.tile([1, 512], FP32, tag="rowacc", name="n_ps")
    nst = 4 * tb + 4
    qks = []
    for g in range(nst + 1):
        if g < nst:
            src = (g // 2) if b == 0 else (7 - g // 2)
            co = (g % 2) * 128 + b * OB
            qk = psq.tile([128, 512], FP32, tag="qk")
            if b == 0:
                nc.tensor.matmul(qk[:], kT_all[:, src, ds(co, 128)],
                                 qT_all[:, ds(2 * tb, 2), 0:OB],
                                 start=True, stop=True)
            else:
                nc.tensor.matmul(qk[:, 0:256], kT_all[:, src, ds(co, 128)],
                                 qT_all[:, 7 - 2 * tb, OB:OT],
                                 start=True, stop=True)
                nc.tensor.matmul(qk[:, 256:512], kT_all[:, src, ds(co, 128)],
                                 qT_all[:, 6 - 2 * tb, OB:OT],
                                 start=True, stop=True)
            qks.append(qk)
        if g >= 1:               # post-process group g-1 (1-deep pipeline)
            gp = g - 1
            src = (gp // 2) if b == 0 else (7 - gp // 2)
            qkp = qks[gp]
            sexp = tp.tile([128, 512], BF16, tag="sexp")
            nc.scalar.activation(sexp[:], fb_ps[:], AF.Exp,
                                 bias=dcolT[:, gp, b:b + 1])
            cp = tp.tile([128, 512], BF16, tag="cp")
            nc.vector.tensor_tensor(cp[:], qkp[:], sexp[:], AL.mult)
            if gp >= 4 * tb:
                kk = gp - 4 * tb
                nc.vector.tensor_tensor(
                    cp[:], cp[:], strip[:, ds((3 - kk) * 128, 512)], AL.mult)
            nc.tensor.matmul(n_ps[:], _G["onesb"][:], cp[:],
                             start=(gp == 0), stop=(gp == nst - 1))
            vi = gp % 2
            nc.tensor.matmul(A0[:], v_b[:, src, vi, ds(0, 128)], cp[:],
                             start=(gp == 0), stop=(gp == nst - 1))
            nc.tensor.matmul(A1[:], v_b[:, src, vi, ds(128, 128)], cp[:],
                             start=(gp == 0), stop=(gp == nst - 1))
    for j in range(2):
        nc.scalar.activation(hout[:, 0, 2 * tb + j], A0[:, ds(j * 256, 256)],
                             AF.Copy)
        nc.scalar.activation(hout[:, 1, 2 * tb + j], A1[:, ds(j * 256, 256)],
                             AF.Copy)
    # inline normalization arg: arg = ssqA/dv + EPS*max(n^2, 1)
    ssq_ps = ps1.tile([1, 512], FP32, tag="rowacc", name="ssq_ps")
    for half in range(2):
        asq = tp.tile([128, 2, 256], BF16, tag="asq")
        nc.vector.tensor_tensor(asq[:], hout[:, half, ds(2 * tb, 2)],
                                hout[:, half, ds(2 * tb, 2)], AL.mult)
        nc.tensor.matmul(ssq_ps[:], _G["onesb"][:], asq[:],
                         start=(half == 0), stop=(half == 1))
    nrow_sb = tp.tile([1, 512], FP32, tag="nrow_sb", name="nrow_sb", bufs=1)
    nc.vector.tensor_copy(nrow_sb[:], n_ps[:])
    nc.vector.tensor_tensor(nrow_sb[:], nrow_sb[:], nrow_sb[:], AL.mult)
    nc.vector.tensor_scalar(nrow_sb[:], nrow_sb[:], 1.0, EPS, AL.max, AL.mult)
    srow_sb = tp.tile([1, 512], FP32, tag="srow_sb", name="srow_sb", bufs=1)
    nc.vector.tensor_copy(srow_sb[:], ssq_ps[:])
    nb = psA.tile([128, 512], FP32, tag="A0", name="nb")
    nc.tensor.matmul(nb[:], _G["ones1f"][0:1, :], nrow_sb[:],
                     start=True, stop=True)
    sb2 = psA.tile([128, 512], FP32, tag="A1", name="sb2")
    nc.tensor.matmul(sb2[:], _G["ones1f"][0:1, :], srow_sb[:],
                     start=True, stop=True)
    s2 = tp.tile([128, 512], FP32, tag="s2", name="s2", bufs=1)
    nc.scalar.activation(s2[:], sb2[:], AF.Copy, scale=1.0 / dv)
    nc.vector.tensor_tensor(argb[:, tb], nb[:], s2[:], AL.add)


def _attn_batch_tail(nc, b, bufs, ps1, tp):
    _attn_tail_compute(nc, b, bufs, tp)
    _attn_tail_fire(nc, b, bufs)


def _attn_tail_compute(nc, b, bufs, tp):
    """Bulk rsqrt + gate/scale hout (no collective interaction)."""
    hout, argb = bufs[f"hout{b}"], bufs[f"argb{b}"]
    # srow = rsqrt(arg), all 4 tb in two ACT ops (in place)
    nc.scalar.activation(argb[:], argb[:], AF.Ln)
    nc.scalar.activation(argb[:], argb[:], AF.Exp, scale=-0.5)
    for tb in range(4):
        for half in range(2):
            for j in range(2):
                hs = hout[:, half, 2 * tb + j]
                nc.vector.tensor_tensor(hs, hs, argb[:, tb, ds(j * 256, 256)],
                                        AL.mult)


def _attn_tail_fire(nc, b, bufs):
    """Payload stores + the hout A2A."""
    rg = bufs["rg"]
    hout = bufs[f"hout{b}"]
    for tb in range(4):
        for hf in range(2):
            dest = (2 * tb + hf) if b == 0 else (7 - 2 * tb - hf)
            nc.scalar.dma_start(
                bufs["a2ah_in"][b][dest].rearrange(
                    "(p half t) -> p half t", p=128, half=2),
                hout[:, :, 2 * tb + hf])
    nc.gpsimd.collective_compute(
        "AllToAll", AL.bypass, replica_groups=rg,
        ins=[bufs["a2ah_in"][b][:]], outs=[bufs["a2ah_out"][b][:]])


def _wout_x1_pieces(nc, b, bufs, wx, wps, mlo, mhi, tp):
    """w_out mix + residual + norm2-ssq for batch-b tokens, m in [mlo,mhi)."""
    wo3 = bufs["wout_d"].rearrange("(kt p) f -> p kt f", p=128)
    if mlo == 0:
        # weight pieces for the first half prefetch BEFORE the collective-
        # gated hA load so the sync queue never blocks them
        wpcs = [wx.tile([128, 16, 128], BF16, tag="wopc", bufs=3,
                        name=f"wpc{b}_{j}") for j in range(3)]
        for j in range(3):
            nc.sync.dma_start(wpcs[j][:], wo3[:, :, ds(j * 128, 128)])
        bufs["wpcs"] = wpcs
        hA = wx.tile([128, 8, 2, 256], BF16, tag="hA", name=f"hA{b}")
        bufs["hA_cur"] = hA
        nc.sync.dma_start(
            hA[:],
            bufs["a2ah_out"][b].rearrange("r (p half t) -> p r half t",
                                          p=128, half=2))
        # sigmoid(own og) gate applied on the receiver side
        nc.vector.tensor_tensor(
            hA[:], hA[:], bufs["sigl"][:, :, :, ds(b * OB, OB)], AL.mult)
        bufs["x1b_cur"] = wx.tile([128, NK, OB], BF16, tag="x1b", name=f"x1b{b}")
        bufs["x1row_ps"] = wps.tile([1, OB], FP32, tag="x1row", name="x1row", bufs=1)
    hA, x1b = bufs["hA_cur"], bufs["x1b_cur"]
    for m in range(mlo, mhi):
        if m < 3 and "wpcs" in bufs:
            wpc = bufs["wpcs"][m]
        else:
            wpc = wx.tile([128, 16, 128], BF16, tag="wopc", bufs=3, name="wpc")
            nc.sync.dma_start(wpc[:], wo3[:, :, ds(m * 128, 128)])
        mps = wps.tile([128, OB], FP32, tag="wops", name="mps", bufs=1)
        for kt in range(16):
            nc.tensor.matmul(mps[:], wpc[:, kt], hA[:, kt // 2, kt % 2],
                             start=(kt == 0), stop=(kt == 15),
                             skip_group_check=True)
        nc.vector.tensor_tensor(x1b[:, m], bufs["xT"][:, m, ds(b * OB, OB)],
                                mps[:], AL.add)
        sqt = tp.tile([128, OB], BF16, tag="sqx")
        nc.vector.tensor_tensor(sqt[:], x1b[:, m], x1b[:, m], AL.mult)
        nc.tensor.matmul(bufs["x1row_ps"][:], _G["onesb"][:], sqt[:],
                         start=(m == 0), stop=(m == 15),
                         skip_group_check=True)


def _wout_x1_tail(nc, b, bufs, wx, wps, tp):
    x1b = bufs["x1b_cur"]
    nc.scalar.dma_start(
        bufs["ox1_d"].rearrange("(kt p) t -> p kt t", p=128)[:, :, ds(b * OB, OB)],
        x1b[:])
    scl = tp.tile([1, OB], FP32, tag="sclx")
    nc.vector.tensor_scalar(scl[:], bufs["x1row_ps"][:], 1.0 / D, EPS,
                            AL.mult, AL.add)
    nc.scalar.activation(scl[:], scl[:], AF.Ln)
    nc.scalar.activation(scl[:], scl[:], AF.Exp, scale=-0.5)
    sc_ps = wps.tile([128, OB], FP32, tag="wops", name="sc_ps", bufs=1)
    nc.tensor.matmul(sc_ps[:], _G["ones1f"][0:1, :], scl[:], start=True, stop=True)
    for kt in range(NK):
        nc.vector.tensor_tensor(x1b[:, kt], x1b[:, kt], sc_ps[:], AL.mult)
    nc.sync.dma_start(
        bufs["ag2_in"][b].rearrange("(kt p) t -> p kt t", p=128), x1b[:])
    nc.gpsimd.collective_compute(
        "AllGather", AL.bypass, replica_groups=bufs["rg"],
        ins=[bufs["ag2_in"][b][:]], outs=[bufs["ag2_out"][b][:]])


def _mlp_pair(nc, b, pair, bufs, hp, tp, psgu, psd):
    """MLP for rb blocks (2*pair, 2*pair+1), batch-b half (2x256 tokens)."""
    h2b = hp.tile([128, NK, OT], BF16, tag="h2b", bufs=1,
                  name=f"h2b_{b}_{pair}")
    for j in range(2):
        nc.sync.dma_start(
            h2b[:, :, ds(j * OB, OB)],
            bufs["ag2_out"][b][2 * pair + j].rearrange("(kt p) t -> p kt t",
                                                       p=128))
    if "wd_t" not in bufs:
        bufs["wd_t"] = hp.tile([128, 8, D], BF16, name="wd_t")
        nc.sync.dma_start(bufs["wd_t"][:],
                          bufs["wd_d"].rearrange("(kt p) f -> p kt f", p=128))
    wg_t, wu_t, wd_t = bufs["wg_t"], bufs["wu_t"], bufs["wd_t"]
    ga = hp.tile([128, 8, OT], BF16, tag="ga", name=f"ga_{b}_{pair}")
    for mf in range(8):
        gps = psgu.tile([128, OT], FP32, tag="g")
        for kt in range(NK):
            nc.tensor.matmul(gps[:], wg_t[:, kt, ds(mf * 128, 128)],
                             h2b[:, kt], start=(kt == 0), stop=(kt == NK - 1))
        nc.scalar.activation(ga[:, mf], gps[:], AF.Silu)
        ups = psgu.tile([128, OT], FP32, tag="u")
        for kt in range(NK):
            nc.tensor.matmul(ups[:], wu_t[:, kt, ds(mf * 128, 128)],
                             h2b[:, kt], start=(kt == 0), stop=(kt == NK - 1))
        # aa = silu(g) * u written in place over ga
        nc.vector.tensor_tensor(ga[:, mf], ups[:], ga[:, mf], AL.mult)
    for tt in range(4):
        rb = 2 * pair + tt // 2
        row0 = rb * OT + b * OB + (tt % 2) * 128
        opss = [psd.tile([128, 512], FP32, tag=f"o{nb}", name=f"o{nb}")
                for nb in range(4)]
        for kt in range(8):
            for nb in range(4):
                nc.tensor.matmul(opss[nb][:], ga[:, kt, ds(tt * 128, 128)],
                                 wd_t[:, kt, ds(nb * 512, 512)],
                                 start=(kt == 0), stop=(kt == 7),
                                 skip_group_check=True)
        for nb in range(4):
            ost = tp.tile([128, 512], BF16, tag="ost", name="ost", bufs=1)
            nc.scalar.activation(ost[:], opss[nb][:], AF.Copy)
            nc.scalar.dma_start(
                bufs["omlp_d"][ds(row0, 128), ds(nb * 512, 512)], ost[:])


def _build():
    nc = bacc.Bacc(num_devices=R)
    rg = [list(range(R))]

    xT_d = nc.dram_tensor("xT", [D, OT], BF16, kind="ExternalInput")
    wcat_d = nc.dram_tensor("wcat", [D, WCOLS], BF16, kind="ExternalInput")
    wv_d = nc.dram_tensor("wv", [D, D], BF16, kind="ExternalInput")
    b16_d = nc.dram_tensor("b16", [16, 1], FP32, kind="ExternalInput")
    wout_d = nc.dram_tensor("wout", [D, D], BF16, kind="ExternalInput")
    wg_d = nc.dram_tensor("wg", [D, FFC], BF16, kind="ExternalInput")
    wu_d = nc.dram_tensor("wu", [D, FFC], BF16, kind="ExternalInput")
    wd_d = nc.dram_tensor("wd", [FFC, D], BF16, kind="ExternalInput")
    strip_d = nc.dram_tensor("strip", [128, 896], BF16, kind="ExternalInput")
    ident_d = nc.dram_tensor("ident", [128, 128], FP32, kind="ExternalInput")
    ones1f_d = nc.dram_tensor("ones1f", [65, 128], FP32, kind="ExternalInput")
    onesb_d = nc.dram_tensor("onesb", [128, 1], BF16, kind="ExternalInput")

    ox1_d = nc.dram_tensor("out_x1", [D, OT], BF16, kind="ExternalOutput")
    omlp_d = nc.dram_tensor("out_mlp", [R * OT, D], BF16, kind="ExternalOutput")

    bufs = {
        "rg": rg, "ox1_d": ox1_d, "omlp_d": omlp_d, "wout_d": wout_d,
        "wg_d": wg_d, "wu_d": wu_d, "wd_d": wd_d,
        "a2a1_in": nc.dram_tensor("a2a1_in", [R, 131072], BF16),
        "a2a1_out": nc.dram_tensor("a2a1_out", [R, 131072], BF16),
        "a2av_in": nc.dram_tensor("a2av_in", [R, 131072], BF16),
        "a2av_out": nc.dram_tensor("a2av_out", [R, 131072], BF16),
        "ag_g_in": nc.dram_tensor("ag_g_in", [R, 2, OT], FP32),
        "ag_g_out": nc.dram_tensor("ag_g_out", [R, 2, OT], FP32),
        "a2ah_in": [nc.dram_tensor(f"a2ah_in{b}", [R, 65536], BF16)
                    for b in range(2)],
        "a2ah_out": [nc.dram_tensor(f"a2ah_out{b}", [R, 65536], BF16)
                     for b in range(2)],
        "ag2_in": [nc.dram_tensor(f"ag2_in{b}", [D, OB], BF16)
                   for b in range(2)],
        "ag2_out": [nc.dram_tensor(f"ag2_out{b}", [R, D, OB], BF16,
                                   addr_space="Shared") for b in range(2)],
    }

    with TileContext(nc) as tc:
        with tc.tile_pool(name="glob", bufs=1) as gp:
            strip = gp.tile([128, 896], BF16)
            nc.sync.dma_start(strip[:], strip_d[:])
            ident = gp.tile([128, 128], FP32)
            nc.sync.dma_start(ident[:], ident_d[:])
            ones1f = gp.tile([65, 128], FP32)
            nc.sync.dma_start(ones1f[:], ones1f_d[:])
            onesb = gp.tile([128, 1], BF16)
            nc.sync.dma_start(onesb[:], onesb_d[:])
            b16 = gp.tile([16, 1], FP32)
            nc.sync.dma_start(b16[:], b16_d[:])
            _G.update(strip=strip, ident=ident, ones1f=ones1f, onesb=onesb)

            # HAM warmup: junk matmuls while xT/weights stream in
            with tc.tile_pool(name="warm", bufs=2, space="PSUM") as wmp:
                for _ in range(24):
                    wps0 = wmp.tile([128, 512], FP32, tag="wm")
                    nc.tensor.matmul(wps0[:], strip[:, 0:128], strip[:, 128:640],
                                     start=True, stop=True)

            with tc.tile_pool(name="mid", bufs=1) as mp:
                xT = mp.tile([128, NK, OT], BF16)
                nc.sync.dma_start(xT[:],
                                  xT_d.rearrange("(kt p) t -> p kt t", p=128))
                bufs["xT"] = xT
                bufs["P2"] = mp.tile([1, 2 * S], FP32, name="P2")
                bufs["dcolT"] = mp.tile([128, 16, 2], FP32, name="dcolT")
                bufs["sigl"] = mp.tile([128, 8, 2, OT], BF16, name="sigl")

                _mixer(nc, tc, xT, wcat_d, wv_d, b16, bufs)

                with tc.tile_pool(name="mlpw", bufs=1) as mwp, \
                     tc.tile_pool(name="wx", bufs=1) as wx, \
                     tc.tile_pool(name="wx_tp", bufs=2) as wxtp, \
                     tc.tile_pool(name="wx_ps", bufs=2, space="PSUM") as wps:
                    with tc.tile_pool(name="at_pay", bufs=1) as ap, \
                         tc.tile_pool(name="at_tmp", bufs=2) as tp, \
                         tc.tile_pool(name="at_psq", bufs=2, space="PSUM") as psq, \
                         tc.tile_pool(name="at_psA", bufs=1, space="PSUM") as psA, \
                         tc.tile_pool(name="at_ps1", bufs=1, space="PSUM") as ps1:
                        qT_all = ap.tile([128, R, OT], BF16)
                        nc.scalar.dma_start(
                            qT_all[:],
                            bufs["a2a1_out"][:, ds(PAY_Q, 65536)].rearrange(
                                "r (p t) -> p r t", p=128))
                        kT_all = ap.tile([128, R, OT], BF16)
                        nc.scalar.dma_start(
                            kT_all[:],
                            bufs["a2a1_out"][:, ds(PAY_K, 65536)].rearrange(
                                "r (p t) -> p r t", p=128))
                        bufs["qT_all"], bufs["kT_all"] = qT_all, kT_all
                        bufs["wg_t"] = mwp.tile([128, NK, FFC], BF16,
                                                name="wg_t")
                        bufs["wu_t"] = mwp.tile([128, NK, FFC], BF16,
                                                name="wu_t")
                        nc.sync.dma_start(
                            bufs["wg_t"][:],
                            bufs["wg_d"].rearrange("(kt p) f -> p kt f", p=128))
                        nc.sync.dma_start(
                            bufs["wu_t"][:],
                            bufs["wu_d"].rearrange("(kt p) f -> p kt f", p=128))

                        _attn_batch_start(nc, 0, bufs, ap)
                        for tb in range(4):
                            _attn_tb(nc, 0, tb, bufs, psq, psA, ps1, tp)
                        _attn_tail_compute(nc, 0, bufs, tp)
                        # b1 payload loads go on the gpsimd queue BEFORE the
                        # hout A2A trigger so they don't wait on it
                        _attn_batch_start(nc, 1, bufs, ap)
                        _attn_tail_fire(nc, 0, bufs)
                        _attn_tb(nc, 1, 0, bufs, psq, psA, ps1, tp)
                        _wout_x1_pieces(nc, 0, bufs, wx, wps, 0, 8, wxtp)
                        _attn_tb(nc, 1, 1, bufs, psq, psA, ps1, tp)
                        _wout_x1_pieces(nc, 0, bufs, wx, wps, 8, 16, wxtp)
                        _wout_x1_tail(nc, 0, bufs, wx, wps, wxtp)
                        _attn_tb(nc, 1, 2, bufs, psq, psA, ps1, tp)
                        _attn_tb(nc, 1, 3, bufs, psq, psA, ps1, tp)
                        _attn_tail_compute(nc, 1, bufs, tp)
                        _attn_tail_fire(nc, 1, bufs)

                    with tc.tile_pool(name="ml_h", bufs=1) as hp, \
                         tc.tile_pool(name="ml_tmp", bufs=2) as mtp, \
                         tc.tile_pool(name="ml_psgu", bufs=1, space="PSUM") as psgu, \
                         tc.tile_pool(name="ml_psd", bufs=1, space="PSUM") as psd:
                        _mlp_pair(nc, 0, 0, bufs, hp, mtp, psgu, psd)
                        _wout_x1_pieces(nc, 1, bufs, wx, wps, 0, 16, wxtp)
                        _wout_x1_tail(nc, 1, bufs, wx, wps, wxtp)
                        for pair in range(1, 4):
                            _mlp_pair(nc, 0, pair, bufs, hp, mtp, psgu, psd)
                        for pair in range(4):
                            _mlp_pair(nc, 1, pair, bufs, hp, mtp, psgu, psd)

    nc.finalize()
    return nc


_NC_CACHE = None


def kernel(x, norm1_w, wq, wk, wv, w_ig, b_ig, w_fg, b_fg, w_og, mh_w,
           w_out, norm2_w, w_gate, w_up, w_down):
    global _NC_CACHE
    x = np.asarray(x, np.float32)
    n1 = np.asarray(norm1_w, np.float32)
    n2 = np.asarray(norm2_w, np.float32)
    mh = np.asarray(mh_w, np.float32)

    wif = np.empty((D, 2 * H), np.float32)
    wif[:, 0::2] = np.asarray(w_ig)
    wif[:, 1::2] = np.asarray(w_fg)
    b16v = np.empty((16, 1), np.float32)
    b16v[0::2, 0] = -2.0 * np.asarray(b_ig) / CAP
    b16v[1::2, 0] = -2.0 * np.asarray(b_fg) / CAP

    wq_s = np.asarray(wq) / np.float32(np.sqrt(dqk))
    wcat = (np.concatenate([wq_s, np.asarray(wk), np.asarray(w_og), wif],
                           axis=1) * n1[:, None]).astype(bf16)
    wv_b = (np.asarray(wv) * n1[:, None]).astype(bf16)
    wout_f = (np.asarray(w_out) * mh[:, None]).astype(bf16)
    wg_f = (np.asarray(w_gate) * n2[:, None]).astype(bf16)
    wu_f = (np.asarray(w_up) * n2[:, None]).astype(bf16)
    wd_b = np.asarray(w_down).astype(bf16)

    i_idx = np.arange(128)[:, None]
    c_idx = np.arange(896)[None, :]
    strip = ((c_idx - i_idx) >= 384).astype(bf16)
    ident = np.eye(128, dtype=np.float32)
    ones1f = np.ones((65, 128), np.float32)
    onesb = np.ones((128, 1), bf16)

    in_maps = []
    for c in range(R):
        s0 = slice(OB * c, OB * (c + 1))
        s1 = slice(OB * (7 - c), OB * (8 - c))
        xT = np.ascontiguousarray(
            np.concatenate([x[0, s0].T, x[1, s1].T], axis=1)).astype(bf16)
        in_maps.append({
            "xT": xT, "wcat": wcat, "wv": wv_b, "b16": b16v,
            "wout": wout_f,
            "wg": np.ascontiguousarray(wg_f[:, FFC * c:FFC * (c + 1)]),
            "wu": np.ascontiguousarray(wu_f[:, FFC * c:FFC * (c + 1)]),
            "wd": np.ascontiguousarray(wd_b[FFC * c:FFC * (c + 1)]),
            "strip": strip, "ident": ident, "ones1f": ones1f, "onesb": onesb,
        })

    if _NC_CACHE is None:
        _NC_CACHE = _build()
    res = run_bass_kernel_spmd(_NC_CACHE, in_maps, core_ids=list(range(R)))

    import os
    if os.environ.get("KDBG"):
        np.savez("/tmp/kdbg.npz",
                 **{f"x1_{c}": np.asarray(res.results[c]["out_x1"])
                    for c in range(R)},
                 **{f"mlp_{c}": np.asarray(res.results[c]["out_mlp"])
                    for c in range(R)})
    out = np.zeros((B, S, D), np.float32)
    for c in range(R):
        x1T = np.asarray(res.results[c]["out_x1"]).astype(np.float32)
        s0 = slice(OB * c, OB * (c + 1))
        s1 = slice(OB * (7 - c), OB * (8 - c))
        out[0, s0] = x1T[:, :OB].T
        out[1, s1] = x1T[:, OB:].T
    mlp = np.zeros((R * OT, D), np.float32)
    for c in range(R):
        mlp += np.asarray(res.results[c]["out_mlp"]).astype(np.float32)
    for r in range(R):
        blk = mlp[r * OT:(r + 1) * OT]
        out[0, OB * r:OB * (r + 1)] += blk[:OB]
        out[1, OB * (7 - r):OB * (8 - r)] += blk[OB:]
    return out
